# revision 35
# baseline (speedup 1.0000x reference)
"""Trainium2 Bass kernel for pairwise-MLP GNN message passing.

dro[b,i,j] = W3^T relu(W2^T relu(PhiA_i + PhiB_j ...) + b2) + b3 with the
first linear layer factorized as hA_i + hB_j.

Sharding: robot-row dimension N=512 split across 8 cores (64 rows each);
all other tensors replicated. Each core computes a [B, 64, N] slab.

Math rewrite (host does only O(H^2) weight prep):
  s_h relu(z2_h) = (z2'_h + |z2'_h|)/2 where z2' = z2 * s (signs folded
  into W2). With W2f = lam/2 * W2 * w3 (+ b2 row for the ones trick) and
  an extra column wtil = W2f @ 1:
    dro[b,i,j] = (sum_h |z2'[j,h]| + z2'[j,320]) / lam + b3
  z2'[j,:] = t1e[:,j]^T @ W2F   (PE; k<256 fp8 DoubleRow, k>=256 bf16)
  t1e[k,j] = relu(hA[b,i,k] + hBT[b][k,j]),  t1e[320,j] = 1

Engine layout per i: ACT 2 fp8 relu (k-chunks 0,1) + most chunk-2 relu;
DVE one [128,4,320] abs-reduce (axis=X) + [128,4] sum-col gather + a
share of chunk-2 relu (bf16 2x mode); PE 4x(DoubleRow + K=65 bf16)
matmuls. Scalar engine carries nothing but relu.
"""

import numpy as np

import concourse.bass as bass
import concourse.mybir as mybir
import concourse.tile as tile
from concourse import bacc
from concourse import bass_utils
from concourse.masks import make_identity

F32 = mybir.dt.float32
F32R = mybir.dt.float32r
BF16 = mybir.dt.bfloat16
FP8 = mybir.dt.float8e4
ALU = mybir.AluOpType
ACTF = mybir.ActivationFunctionType
PM = mybir.MatmulPerfMode

B, N, E, L = 2, 512, 128, 32
D = E + L            # 160
H = 2 * D            # 320
HP = H + 1           # 321 = H plus the linear-term column
NCORES = 8
NI = N // NCORES     # 64 robot rows per core
MS = [(0, 128), (128, 128), (256, 64)]   # m-tiles of H (hA/hB build)
NJT = 4                                   # j-tiles of 128
LAM = 256.0                               # fp8 weight scale
DVE_L1 = 3            # chunk-2 relu runs on DVE when i % DVE_L1 == 0

_CACHE = {}


def _build(P):
    nc = bacc.Bacc("TRN2", target_bir_lowering=False, debug=False,
                   enable_asserts=False, num_devices=NCORES)

    robot = nc.dram_tensor("robot", [B, NI, E], F32, kind="ExternalInput").ap()
    obj = nc.dram_tensor("obj", [B, N, E], F32, kind="ExternalInput").ap()
    W1A = nc.dram_tensor("W1A", [E, H], F32, kind="ExternalInput").ap()
    W1B = nc.dram_tensor("W1B", [E, H], F32, kind="ExternalInput").ap()
    zAT = nc.dram_tensor("zAT", [H, B], F32, kind="ExternalInput").ap()
    zBT = nc.dram_tensor("zBT", [H, B], F32, kind="ExternalInput").ap()
    # k-chunks 0..1 as [p, d, n] = W2e[p+128d, n] (bf16)
    W2dr = nc.dram_tensor("W2dr", [128, 2, H], BF16, kind="ExternalInput").ap()
    # k-chunk 2 (rows 256..320 incl ones row) in bf16
    W2c2 = nc.dram_tensor("W2c2", [65, H], BF16, kind="ExternalInput").ap()
    signs = nc.dram_tensor("signs", [128, H], F32, kind="ExternalInput").ap()
    lb3 = nc.dram_tensor("lb3", [128, 1], F32, kind="ExternalInput").ap()
    out = nc.dram_tensor("out", [B, NI, N], F32, kind="ExternalOutput").ap()

    with tile.TileContext(nc) as tc:
        with tc.tile_pool(name="persist", bufs=1) as pp:
            # ---- persistent tiles ----
            ident = pp.tile([128, 128], F32, tag="ident")
            make_identity(nc, ident[:])
            # force the ACT function-table load early so it overlaps setup
            warm = pp.tile([1, 1], F32, tag="warm")
            nc.scalar.activation(warm[:], ident[0:1, 0:1], ACTF.Relu)
            b3t = pp.tile([128, 1], F32, tag="b3t")
            nc.gpsimd.dma_start(b3t[:], lb3)
            sg = pp.tile([128, H], F32, tag="sg")
            nc.sync.dma_start(sg[:], signs)
            w2dr = pp.tile([128, 2, H], BF16, tag="w2dr")
            nc.sync.dma_start(w2dr[:], W2dr)
            w2c2 = pp.tile([65, H], BF16, tag="w2c2")
            nc.gpsimd.dma_start(w2c2[:], W2c2)
            # f32r W1 tiles (must be produced by a compute engine)
            with tc.tile_pool(name="wstg", bufs=2) as wstg:
                stg = wstg.tile([E, H], F32, tag="wstg")
                nc.sync.dma_start(stg[:], W1A)
                w1a = pp.tile([E, H], F32R, tag="w1a")
                nc.vector.tensor_copy(w1a[:], stg[:])
                stg = wstg.tile([E, H], F32, tag="wstg")
                nc.gpsimd.dma_start(stg[:], W1B)
                w1b = pp.tile([E, H], F32R, tag="w1b")
                nc.vector.tensor_copy(w1b[:], stg[:])
            zat, zbt = [], []
            for m, (m0, sz) in enumerate(MS):
                t = pp.tile([sz, B], F32, tag=f"zat_{m}")
                nc.sync.dma_start(t[:], zAT[m0:m0 + sz, :])
                zat.append(t)
                t = pp.tile([sz, B], F32, tag=f"zbt_{m}")
                nc.gpsimd.dma_start(t[:], zBT[m0:m0 + sz, :])
                zbt.append(t)

            hbt = {}  # (b, m) -> bf16 [szk, N]; m=2 has ones row at 64
            hat = {}  # (b, m) -> f32 [szk, NI]; m=2 has zeros row at 64

            # ---- setup: build hA^T, hB^T on device ----
            with tc.tile_pool(name="s_sb", bufs=2) as ssb, \
                 tc.tile_pool(name="s_ps", bufs=2, space="PSUM") as sps:
                for b in range(B):
                    # hB^T[b]: [H, N] from obj[b] @ W1B (+ zB bias)
                    objT_ps = sps.tile([128, N], F32, tag="objT_ps")
                    for jt in range(NJT):
                        stg = ssb.tile([128, E], F32, tag="stg", bufs=2)
                        qs = [nc.sync, nc.gpsimd, nc.sync, nc.gpsimd]
                        qs[jt].dma_start(
                            stg[:], obj[b, jt * 128:(jt + 1) * 128, :])
                        nc.tensor.transpose(objT_ps[:, jt * 128:(jt + 1) * 128],
                                            stg[:], ident[:])
                    objT = ssb.tile([128, N], F32R, tag="objT")
                    nc.vector.tensor_copy(objT[:], objT_ps[:])
                    for m, (m0, sz) in enumerate(MS):
                        hps = sps.tile([sz, N], F32, tag="hps")
                        nc.tensor.matmul(hps[:], w1b[:, m0:m0 + sz], objT[:],
                                         start=True, stop=True)
                        szk = 65 if m == 2 else 128
                        t = pp.tile([szk, N], BF16, tag=f"hbt_{b}_{m}")
                        nc.vector.tensor_scalar(out=t[0:sz, :], in0=hps[:],
                                                scalar1=zbt[m][:, b:b + 1],
                                                scalar2=None, op0=ALU.add)
                        if m == 2:
                            nc.gpsimd.memset(t[64:65, :], 1.0)
                        hbt[(b, m)] = t

                    # hA^T[b]: [H, NI] from robot[b] @ W1A (+ zA bias)
                    stg2 = ssb.tile([NI, E], F32, tag="stg2")
                    (nc.sync if b == 0 else nc.gpsimd).dma_start(
                        stg2[:], robot[b, :, :])
                    robT_ps = sps.tile([128, NI], F32, tag="robT_ps")
                    nc.tensor.transpose(robT_ps[:], stg2[:], ident[0:NI, 0:NI])
                    robT = ssb.tile([128, NI], F32R, tag="robT")
                    nc.vector.tensor_copy(robT[:], robT_ps[:])
                    for m, (m0, sz) in enumerate(MS):
                        aps_ = sps.tile([sz, NI], F32, tag="aps")
                        nc.tensor.matmul(aps_[:], w1a[:, m0:m0 + sz], robT[:],
                                         start=True, stop=True)
                        szk = 65 if m == 2 else 128
                        t = pp.tile([szk, NI], F32, tag=f"hat_{b}_{m}")
                        nc.vector.tensor_scalar(out=t[0:sz, :], in0=aps_[:],
                                                scalar1=zat[m][:, b:b + 1],
                                                scalar2=None, op0=ALU.add)
                        if m == 2:
                            nc.gpsimd.memset(t[64:65, :], 0.0)
                        hat[(b, m)] = t

            # ---- main loop ----
            with tc.tile_pool(name="t1p", bufs=4) as t1p, \
                 tc.tile_pool(name="z2p", bufs=2, space="PSUM") as z2p, \
                 tc.tile_pool(name="scrp", bufs=4) as scrp, \
                 tc.tile_pool(name="accp", bufs=2) as accp, \
                 tc.tile_pool(name="outp", bufs=2) as outp:
                for b in range(B):
                    # per-(i,jt) scalars: [j-part, jt, i]
                    osig = accp.tile([128, 4, NI], F32, tag="osig",
                                     name=f"osig_{b}")

                    for i in range(NI):
                        # L1: t1_k = relu(hBT_k + hA_col); k<256 fp8 on ACT,
                        # chunk 2 (65 rows incl ones) bf16 on ACT or DVE.
                        t01 = t1p.tile([128, 2 * N], BF16, tag="t01")
                        for k in range(2):
                            nc.scalar.activation(
                                t01[:, k * N:(k + 1) * N], hbt[(b, k)][:],
                                ACTF.Relu, bias=hat[(b, k)][:, i:i + 1])
                        t2 = t1p.tile([65, N], BF16, tag="t2")
                        if i % DVE_L1 == 0:
                            nc.vector.tensor_scalar(
                                out=t2[:], in0=hbt[(b, 2)][:],
                                scalar1=hat[(b, 2)][:, i:i + 1],
                                scalar2=0.0, op0=ALU.add, op1=ALU.max)
                        else:
                            nc.scalar.activation(
                                t2[:], hbt[(b, 2)][:], ACTF.Relu,
                                bias=hat[(b, 2)][:, i:i + 1])
                        # L2 + L3 per j-tile: z2[jt] = t1^T @ W2e, then fused
                        # relu*signs + row-reduce on DVE.
                        for jt in range(NJT):
                            jc = jt * 128
                            zt = z2p.tile([128, H], F32, tag=f"z2_{jt}")
                            nc.tensor.matmul(zt[:], t01[:, jc:jc + 128],
                                             w2dr[:, 0, :], start=True,
                                             stop=False)
                            nc.tensor.matmul(zt[:], t01[:, N + jc:N + jc + 128],
                                             w2dr[:, 1, :], start=False,
                                             stop=False)
                            nc.tensor.matmul(zt[:], t2[:, jc:jc + 128],
                                             w2c2[:], start=False, stop=True)
                            s = scrp.tile([128, H], F32, tag="scr_d")
                            nc.vector.scalar_tensor_tensor(
                                out=s[:], in0=zt[:], scalar=0.0,
                                in1=sg[:], op0=ALU.max, op1=ALU.mult,
                                accum_out=osig[:, jt, i:i + 1])

                    # epilogue for batch b: (|.|sum + lin)/lam, transpose, store
                    osb = outp.tile([NI, N], F32, tag="osb")
                    for jt in range(NJT):
                        oc = outp.tile([128, NI], F32, tag=f"oc_{jt % 2}")
                        nc.vector.tensor_scalar(
                            out=oc[:], in0=osig[:, jt, :],
                            scalar1=b3t[:, 0:1], scalar2=None, op0=ALU.add)
                        tp = z2p.tile([NI, 128], F32, tag=f"z2_{jt}")
                        nc.tensor.transpose(tp[:], oc[:], ident[:])
                        nc.vector.tensor_copy(osb[:, jt * 128:(jt + 1) * 128],
                                              tp[:])
                    nc.sync.dma_start(out[b, :, :], osb[:])

    nc.compile()
    return nc


def _prep(robot_embedding_tf, object_embedding_tf, z, W1, b1, W2, b2, W3, b3):
    """Host-side weight prep (O(H^2)) + per-core input maps."""
    import ml_dtypes
    f = np.float32
    robot = np.ascontiguousarray(robot_embedding_tf, dtype=f)
    obj = np.ascontiguousarray(object_embedding_tf, dtype=f)
    z = np.asarray(z, dtype=f)
    W1 = np.asarray(W1, dtype=f)
    b1 = np.asarray(b1, dtype=f)
    W2 = np.asarray(W2, dtype=f)
    b2 = np.asarray(b2, dtype=f)
    W3 = np.asarray(W3, dtype=f)
    b3 = np.asarray(b3, dtype=f)

    w3 = W3[:, 0]
    aw3 = np.abs(w3)
    s = np.sign(w3)
    W2F = np.vstack([W2 * aw3[None, :], (b2 * aw3)[None, :]])  # [321,320]
    P = 0
    W2dr = np.empty((128, 2, H), dtype=ml_dtypes.bfloat16)
    W2dr[:, 0, :] = W2F[0:128, :].astype(ml_dtypes.bfloat16)
    W2dr[:, 1, :] = W2F[128:256, :].astype(ml_dtypes.bfloat16)
    W2c2 = np.ascontiguousarray(W2F[256:321, :].astype(ml_dtypes.bfloat16))
    signs = np.ascontiguousarray(np.broadcast_to(s[None, :], (128, H)), dtype=f)
    lb3 = np.full((128, 1), b3[0], dtype=f)

    zA = z @ W1[E:D, :]                 # [B, H]
    zB = z @ W1[D + E:, :] + b1[None, :]
    zAT = np.ascontiguousarray(zA.T, dtype=f)
    zBT = np.ascontiguousarray(zB.T, dtype=f)
    W1A = np.ascontiguousarray(W1[0:E, :], dtype=f)
    W1B = np.ascontiguousarray(W1[D:D + E, :], dtype=f)

    shared = dict(obj=obj, W1A=W1A, W1B=W1B, zAT=zAT, zBT=zBT, W2dr=W2dr,
                  W2c2=W2c2, signs=signs, lb3=lb3)
    in_maps = []
    for c in range(NCORES):
        m = dict(shared)
        m["robot"] = np.ascontiguousarray(robot[:, c * NI:(c + 1) * NI, :])
        in_maps.append(m)
    return in_maps, P


def _run(trace=False, **inputs):
    in_maps, P = _prep(**inputs)
    if _CACHE.get("P") != P:
        _CACHE["nc"] = _build(P)
        _CACHE["P"] = P
    nc = _CACHE["nc"]
    res = bass_utils.run_bass_kernel_spmd(
        nc, in_maps, core_ids=list(range(NCORES)), trace=trace)
    dro = np.empty((B, N, N), dtype=np.float32)
    for c in range(NCORES):
        dro[:, c * NI:(c + 1) * NI, :] = res.results[c]["out"]
    return dro, res


def kernel(**inputs) -> np.ndarray:
    dro, _ = _run(trace=False, **inputs)
    return dro


# revision 37
# speedup vs baseline: 1.0539x; 1.0539x over previous
"""Trainium2 Bass kernel for pairwise-MLP GNN message passing.

dro[b,i,j] = W3^T relu(W2^T relu(PhiA_i + PhiB_j ...) + b2) + b3 with the
first linear layer factorized as hA_i + hB_j.

Sharding: robot-row dimension N=512 split across 8 cores (64 rows each);
all other tensors replicated. Each core computes a [B, 64, N] slab.

Math rewrite (host does only O(H^2) weight prep):
  s_h relu(z2_h) = (z2'_h + |z2'_h|)/2 where z2' = z2 * s (signs folded
  into W2). With W2f = lam/2 * W2 * w3 (+ b2 row for the ones trick) and
  an extra column wtil = W2f @ 1:
    dro[b,i,j] = (sum_h |z2'[j,h]| + z2'[j,320]) / lam + b3
  z2'[j,:] = t1e[:,j]^T @ W2F   (PE; k<256 fp8 DoubleRow, k>=256 bf16)
  t1e[k,j] = relu(hA[b,i,k] + hBT[b][k,j]),  t1e[320,j] = 1

Engine layout per i: ACT 2 fp8 relu (k-chunks 0,1) + most chunk-2 relu;
DVE one [128,4,320] abs-reduce (axis=X) + [128,4] sum-col gather + a
share of chunk-2 relu (bf16 2x mode); PE 4x(DoubleRow + K=65 bf16)
matmuls. Scalar engine carries nothing but relu.
"""

import numpy as np

import concourse.bass as bass
import concourse.mybir as mybir
import concourse.tile as tile
from concourse import bacc
from concourse import bass_utils
from concourse.masks import make_identity

F32 = mybir.dt.float32
F32R = mybir.dt.float32r
BF16 = mybir.dt.bfloat16
FP8 = mybir.dt.float8e4
ALU = mybir.AluOpType
ACTF = mybir.ActivationFunctionType
PM = mybir.MatmulPerfMode

B, N, E, L = 2, 512, 128, 32
D = E + L            # 160
H = 2 * D            # 320
HP = H + 1           # 321 = H plus the linear-term column
NCORES = 8
NI = N // NCORES     # 64 robot rows per core
MS = [(0, 128), (128, 128), (256, 64)]   # m-tiles of H (hA/hB build)
NJT = 4                                   # j-tiles of 128
LAM = 256.0                               # fp8 weight scale
DVE_L1 = (0, 2)       # chunk-2 relu runs on DVE when i % 5 is in this set

_CACHE = {}


def _build(P):
    nc = bacc.Bacc("TRN2", target_bir_lowering=False, debug=False,
                   enable_asserts=False, num_devices=NCORES)

    robot = nc.dram_tensor("robot", [B, NI, E], F32, kind="ExternalInput").ap()
    obj = nc.dram_tensor("obj", [B, N, E], F32, kind="ExternalInput").ap()
    W1A = nc.dram_tensor("W1A", [E, H], F32, kind="ExternalInput").ap()
    W1B = nc.dram_tensor("W1B", [E, H], F32, kind="ExternalInput").ap()
    zAT = nc.dram_tensor("zAT", [H, B], F32, kind="ExternalInput").ap()
    zBT = nc.dram_tensor("zBT", [H, B], F32, kind="ExternalInput").ap()
    # k-chunks 0..1 as [p, d, n] = W2e[p+128d, n] (bf16)
    W2dr = nc.dram_tensor("W2dr", [128, 2, H], BF16, kind="ExternalInput").ap()
    # k-chunk 2 (rows 256..320 incl ones row) in bf16
    W2c2 = nc.dram_tensor("W2c2", [65, H], BF16, kind="ExternalInput").ap()
    signs = nc.dram_tensor("signs", [128, H], F32, kind="ExternalInput").ap()
    lb3 = nc.dram_tensor("lb3", [128, 1], F32, kind="ExternalInput").ap()
    out = nc.dram_tensor("out", [B, NI, N], F32, kind="ExternalOutput").ap()

    with tile.TileContext(nc) as tc:
        with tc.tile_pool(name="persist", bufs=1) as pp:
            # ---- persistent tiles ----
            ident = pp.tile([128, 128], F32, tag="ident")
            make_identity(nc, ident[:])
            # force the ACT function-table load early so it overlaps setup
            warm = pp.tile([1, 1], F32, tag="warm")
            nc.scalar.activation(warm[:], ident[0:1, 0:1], ACTF.Relu)
            b3t = pp.tile([128, 1], F32, tag="b3t")
            nc.gpsimd.dma_start(b3t[:], lb3)
            sg = pp.tile([128, H], F32, tag="sg")
            nc.sync.dma_start(sg[:], signs)
            w2dr = pp.tile([128, 2, H], BF16, tag="w2dr")
            nc.sync.dma_start(w2dr[:], W2dr)
            w2c2 = pp.tile([65, H], BF16, tag="w2c2")
            nc.gpsimd.dma_start(w2c2[:], W2c2)
            # f32r W1 tiles (must be produced by a compute engine)
            with tc.tile_pool(name="wstg", bufs=2) as wstg:
                stg = wstg.tile([E, H], F32, tag="wstg")
                nc.sync.dma_start(stg[:], W1A)
                w1a = pp.tile([E, H], F32R, tag="w1a")
                nc.vector.tensor_copy(w1a[:], stg[:])
                stg = wstg.tile([E, H], F32, tag="wstg")
                nc.gpsimd.dma_start(stg[:], W1B)
                w1b = pp.tile([E, H], F32R, tag="w1b")
                nc.vector.tensor_copy(w1b[:], stg[:])
            zat, zbt = [], []
            for m, (m0, sz) in enumerate(MS):
                t = pp.tile([sz, B], F32, tag=f"zat_{m}")
                nc.sync.dma_start(t[:], zAT[m0:m0 + sz, :])
                zat.append(t)
                t = pp.tile([sz, B], F32, tag=f"zbt_{m}")
                nc.gpsimd.dma_start(t[:], zBT[m0:m0 + sz, :])
                zbt.append(t)

            hbt = {}  # (b, m) -> bf16 [szk, N]; m=2 has ones row at 64
            hat = {}  # (b, m) -> f32 [szk, NI]; m=2 has zeros row at 64

            # ---- setup: build hA^T, hB^T on device ----
            with tc.tile_pool(name="s_sb", bufs=2) as ssb, \
                 tc.tile_pool(name="s_ps", bufs=2, space="PSUM") as sps:
                for b in range(B):
                    # hB^T[b]: [H, N] from obj[b] @ W1B (+ zB bias)
                    objT_ps = sps.tile([128, N], F32, tag="objT_ps")
                    for jt in range(NJT):
                        stg = ssb.tile([128, E], F32, tag="stg", bufs=2)
                        qs = [nc.sync, nc.gpsimd, nc.sync, nc.gpsimd]
                        qs[jt].dma_start(
                            stg[:], obj[b, jt * 128:(jt + 1) * 128, :])
                        nc.tensor.transpose(objT_ps[:, jt * 128:(jt + 1) * 128],
                                            stg[:], ident[:])
                    objT = ssb.tile([128, N], F32R, tag="objT")
                    nc.vector.tensor_copy(objT[:], objT_ps[:])
                    for m, (m0, sz) in enumerate(MS):
                        hps = sps.tile([sz, N], F32, tag="hps")
                        nc.tensor.matmul(hps[:], w1b[:, m0:m0 + sz], objT[:],
                                         start=True, stop=True)
                        szk = 65 if m == 2 else 128
                        t = pp.tile([szk, N], BF16, tag=f"hbt_{b}_{m}")
                        nc.vector.tensor_scalar(out=t[0:sz, :], in0=hps[:],
                                                scalar1=zbt[m][:, b:b + 1],
                                                scalar2=None, op0=ALU.add)
                        if m == 2:
                            nc.gpsimd.memset(t[64:65, :], 1.0)
                        hbt[(b, m)] = t

                    # hA^T[b]: [H, NI] from robot[b] @ W1A (+ zA bias)
                    stg2 = ssb.tile([NI, E], F32, tag="stg2")
                    (nc.sync if b == 0 else nc.gpsimd).dma_start(
                        stg2[:], robot[b, :, :])
                    robT_ps = sps.tile([128, NI], F32, tag="robT_ps")
                    nc.tensor.transpose(robT_ps[:], stg2[:], ident[0:NI, 0:NI])
                    robT = ssb.tile([128, NI], F32R, tag="robT")
                    nc.vector.tensor_copy(robT[:], robT_ps[:])
                    for m, (m0, sz) in enumerate(MS):
                        aps_ = sps.tile([sz, NI], F32, tag="aps")
                        nc.tensor.matmul(aps_[:], w1a[:, m0:m0 + sz], robT[:],
                                         start=True, stop=True)
                        szk = 65 if m == 2 else 128
                        t = pp.tile([szk, NI], F32, tag=f"hat_{b}_{m}")
                        nc.vector.tensor_scalar(out=t[0:sz, :], in0=aps_[:],
                                                scalar1=zat[m][:, b:b + 1],
                                                scalar2=None, op0=ALU.add)
                        if m == 2:
                            nc.gpsimd.memset(t[64:65, :], 0.0)
                        hat[(b, m)] = t

            # ---- main loop ----
            with tc.tile_pool(name="t1p", bufs=4) as t1p, \
                 tc.tile_pool(name="z2p", bufs=2, space="PSUM") as z2p, \
                 tc.tile_pool(name="scrp", bufs=4) as scrp, \
                 tc.tile_pool(name="accp", bufs=2) as accp, \
                 tc.tile_pool(name="outp", bufs=2) as outp:
                for b in range(B):
                    # per-(i,jt) scalars: [j-part, jt, i]
                    osig = accp.tile([128, 4, NI], F32, tag="osig",
                                     name=f"osig_{b}")

                    for i in range(NI):
                        # L1: t1_k = relu(hBT_k + hA_col); k<256 fp8 on ACT,
                        # chunk 2 (65 rows incl ones) bf16 on ACT or DVE.
                        t01 = t1p.tile([128, 2 * N], BF16, tag="t01")
                        for k in range(2):
                            nc.scalar.activation(
                                t01[:, k * N:(k + 1) * N], hbt[(b, k)][:],
                                ACTF.Relu, bias=hat[(b, k)][:, i:i + 1])
                        t2 = t1p.tile([65, N], BF16, tag="t2")
                        if i % 5 in DVE_L1:
                            nc.vector.tensor_scalar(
                                out=t2[:], in0=hbt[(b, 2)][:],
                                scalar1=hat[(b, 2)][:, i:i + 1],
                                scalar2=0.0, op0=ALU.add, op1=ALU.max)
                        else:
                            nc.scalar.activation(
                                t2[:], hbt[(b, 2)][:], ACTF.Relu,
                                bias=hat[(b, 2)][:, i:i + 1])
                        # L2 + L3 per j-tile: z2[jt] = t1^T @ W2e, then fused
                        # relu*signs + row-reduce on DVE.
                        for jt in range(NJT):
                            jc = jt * 128
                            zt = z2p.tile([128, H], F32, tag=f"z2_{jt}")
                            nc.tensor.matmul(zt[:], t01[:, jc:jc + 128],
                                             w2dr[:, 0, :], start=True,
                                             stop=False)
                            nc.tensor.matmul(zt[:], t01[:, N + jc:N + jc + 128],
                                             w2dr[:, 1, :], start=False,
                                             stop=False)
                            nc.tensor.matmul(zt[:], t2[:, jc:jc + 128],
                                             w2c2[:], start=False, stop=True)
                            s = scrp.tile([128, H], F32, tag="scr_d")
                            nc.vector.scalar_tensor_tensor(
                                out=s[:], in0=zt[:], scalar=0.0,
                                in1=sg[:], op0=ALU.max, op1=ALU.mult,
                                accum_out=osig[:, jt, i:i + 1])

                    # epilogue for batch b: (|.|sum + lin)/lam, transpose, store
                    osb = outp.tile([NI, N], F32, tag="osb")
                    for jt in range(NJT):
                        oc = outp.tile([128, NI], F32, tag=f"oc_{jt % 2}")
                        nc.vector.tensor_scalar(
                            out=oc[:], in0=osig[:, jt, :],
                            scalar1=b3t[:, 0:1], scalar2=None, op0=ALU.add)
                        tp = z2p.tile([NI, 128], F32, tag=f"z2_{jt}")
                        nc.tensor.transpose(tp[:], oc[:], ident[:])
                        nc.vector.tensor_copy(osb[:, jt * 128:(jt + 1) * 128],
                                              tp[:])
                    nc.sync.dma_start(out[b, :, :], osb[:])

    nc.compile()
    return nc


def _prep(robot_embedding_tf, object_embedding_tf, z, W1, b1, W2, b2, W3, b3):
    """Host-side weight prep (O(H^2)) + per-core input maps."""
    import ml_dtypes
    f = np.float32
    robot = np.ascontiguousarray(robot_embedding_tf, dtype=f)
    obj = np.ascontiguousarray(object_embedding_tf, dtype=f)
    z = np.asarray(z, dtype=f)
    W1 = np.asarray(W1, dtype=f)
    b1 = np.asarray(b1, dtype=f)
    W2 = np.asarray(W2, dtype=f)
    b2 = np.asarray(b2, dtype=f)
    W3 = np.asarray(W3, dtype=f)
    b3 = np.asarray(b3, dtype=f)

    w3 = W3[:, 0]
    aw3 = np.abs(w3)
    s = np.sign(w3)
    W2F = np.vstack([W2 * aw3[None, :], (b2 * aw3)[None, :]])  # [321,320]
    P = 0
    W2dr = np.empty((128, 2, H), dtype=ml_dtypes.bfloat16)
    W2dr[:, 0, :] = W2F[0:128, :].astype(ml_dtypes.bfloat16)
    W2dr[:, 1, :] = W2F[128:256, :].astype(ml_dtypes.bfloat16)
    W2c2 = np.ascontiguousarray(W2F[256:321, :].astype(ml_dtypes.bfloat16))
    signs = np.ascontiguousarray(np.broadcast_to(s[None, :], (128, H)), dtype=f)
    lb3 = np.full((128, 1), b3[0], dtype=f)

    zA = z @ W1[E:D, :]                 # [B, H]
    zB = z @ W1[D + E:, :] + b1[None, :]
    zAT = np.ascontiguousarray(zA.T, dtype=f)
    zBT = np.ascontiguousarray(zB.T, dtype=f)
    W1A = np.ascontiguousarray(W1[0:E, :], dtype=f)
    W1B = np.ascontiguousarray(W1[D:D + E, :], dtype=f)

    shared = dict(obj=obj, W1A=W1A, W1B=W1B, zAT=zAT, zBT=zBT, W2dr=W2dr,
                  W2c2=W2c2, signs=signs, lb3=lb3)
    in_maps = []
    for c in range(NCORES):
        m = dict(shared)
        m["robot"] = np.ascontiguousarray(robot[:, c * NI:(c + 1) * NI, :])
        in_maps.append(m)
    return in_maps, P


def _run(trace=False, **inputs):
    in_maps, P = _prep(**inputs)
    if _CACHE.get("P") != P:
        _CACHE["nc"] = _build(P)
        _CACHE["P"] = P
    nc = _CACHE["nc"]
    res = bass_utils.run_bass_kernel_spmd(
        nc, in_maps, core_ids=list(range(NCORES)), trace=trace)
    dro = np.empty((B, N, N), dtype=np.float32)
    for c in range(NCORES):
        dro[:, c * NI:(c + 1) * NI, :] = res.results[c]["out"]
    return dro, res


def kernel(**inputs) -> np.ndarray:
    dro, _ = _run(trace=False, **inputs)
    return dro
